# revision 2
# baseline (speedup 1.0000x reference)
"""GCN (4x GCNConv + eval BN + ReLU, global mean pool, 2-layer MLP head) on 8
Trainium2 NeuronCores via Bass/Tile.

Sharding: data-parallel over graphs. 4096 graphs -> 8 cores x 512 contiguous
graphs (batch is sorted). Within a core the 512 graphs form 4 pool groups of
128 graphs; each group's nodes are padded to a multiple of 128 rows so pooling
blocks align with node blocks. Edges live on the core owning their dst node.

Per layer (all on device):
  tt = dinv * (h_local @ W_l)           per-core shard, f16 table
  AllGather tt across the 8 cores       (the only collective)
  agg[v] = dinv[v] * sum_{e: dst=v} tt[src_e]   with self-loops as plain edges
  h = BN_l(relu(agg + b_l))
The segment-sum runs as one-hot matmuls: chunks of 128 dst-sorted edges are
gathered from the tt table by indirect DMA, lhsT = is_equal(dst_local, iota),
and the PE accumulates chunks into PSUM per 128-node dst block.

The symmetric GCN normalization dinv[src]*dinv[dst] is folded: dinv[src] into
the table, dinv[dst] into the block epilogue; the self-loop term t*1/deg is
exactly a self-edge under this folding.

All data-dependent structure is precomputed host-side into per-core meta
arrays; the device program is identical across cores (SPMD).
"""

import os
import numpy as np

import concourse.bass as bass
import concourse.tile as tile
from concourse import mybir, bacc, bass_utils
from concourse.masks import make_identity

P = 128
H = 128
N_CORES = 8
N_GRAPHS = 4096
GPC = N_GRAPHS // N_CORES      # graphs per core
GB = 4                         # pool groups (of 128 graphs) per core
BN_EPS = 1e-5
UNROLL = 8                     # chunk buffer sets per stream
NW = 4                         # interleaved block streams

F32 = mybir.dt.float32
F16 = mybir.dt.float16
I32 = mybir.dt.int32

LAST_EXEC_NS = None
_CACHE = {}


def _preprocess(x, src, dst, batch, dinv):
    """Host-side sharding: node remap + per-core padded meta arrays."""
    N = x.shape[0]
    graph_start = np.searchsorted(batch, np.arange(N_GRAPHS + 1))
    seg_rows = np.zeros((N_CORES, GB), dtype=np.int64)
    for c in range(N_CORES):
        for g in range(GB):
            g0 = c * GPC + g * P
            seg_rows[c, g] = graph_start[g0 + P] - graph_start[g0]
    C2 = int(np.ceil(seg_rows.max() / P))     # node blocks per pool group
    NBLK = GB * C2                            # node blocks per core
    NPC = NBLK * P                            # padded nodes per core

    newid = np.zeros(N, dtype=np.int64)
    for c in range(N_CORES):
        for g in range(GB):
            g0 = c * GPC + g * P
            r0, r1 = graph_start[g0], graph_start[g0 + P]
            newid[r0:r1] = c * NPC + g * C2 * P + np.arange(r1 - r0)

    x_loc = np.zeros((N_CORES, NPC, H), dtype=np.float32)
    dinvb = np.ones((N_CORES, P, NBLK), dtype=np.float32)
    glocb = np.full((N_CORES, P, NBLK), -1.0, dtype=np.float32)
    invcnt = np.ones((N_CORES, P, GB), dtype=np.float32)
    loc_all = newid % NPC
    core_all = newid // NPC
    for c in range(N_CORES):
        m = core_all == c
        loc = loc_all[m]
        x_loc[c, loc] = x[m]
        dinvb[c, loc % P, loc // P] = dinv[m]
        gl = (batch[m] - c * GPC).astype(np.int64)      # 0..GPC-1
        glocb[c, loc % P, loc // P] = (gl % P).astype(np.float32)
        cnt = np.zeros(GPC, dtype=np.float64)
        np.add.at(cnt, gl, 1.0)
        invcnt[c] = (1.0 / np.maximum(cnt, 1.0)).reshape(GB, P).T.astype(np.float32)

    # edges + self-loops grouped by dst block.
    # table rows live in [quarter][core][row] order (quarter AllGathers).
    NPQ = NPC // GB
    def table_row(gid):
        c = gid // NPC
        i = gid % NPC
        return (i // NPQ) * (N_CORES * NPQ) + c * NPQ + (i % NPQ)
    e_src_g = table_row(newid[src])
    e_dst_core = core_all[dst]
    e_dst_loc = loc_all[dst]
    counts = np.zeros((N_CORES, NBLK), dtype=np.int64)
    np.add.at(counts, (e_dst_core, e_dst_loc // P), 1)
    C_b = np.maximum(np.ceil(counts.max(axis=0) / P).astype(np.int64), 1)
    col_base = np.concatenate([[0], np.cumsum(C_b)])
    NCH = int(col_base[-1])

    srcg = np.zeros((N_CORES, P, NCH), dtype=np.int32)
    dstl = np.full((N_CORES, P, NCH), -1.0, dtype=np.float32)
    for c in range(N_CORES):
        m = e_dst_core == c
        es = e_src_g[m]
        ed = e_dst_loc[m]
        order = np.argsort(ed // P, kind="stable")
        es, ed = es[order], ed[order]
        blk = ed // P
        blk_starts = np.searchsorted(blk, np.arange(NBLK))
        slot = np.arange(len(es)) - blk_starts[blk]
        col = col_base[blk] + slot // P
        row = slot % P
        srcg[c, row, col] = es.astype(np.int32)
        dstl[c, row, col] = (ed % P).astype(np.float32)

    return dict(C2=C2, NBLK=NBLK, NPC=NPC, NCH=NCH,
                C_b=tuple(int(v) for v in C_b),
                x_loc=x_loc, dinvb=dinvb, glocb=glocb, invcnt=invcnt,
                srcg=srcg, dstl=dstl)


def _build(C2, NBLK, NPC, C_b, NCH, hb2_val):
    col_base = [0]
    for v in C_b:
        col_base.append(col_base[-1] + v)
    table_dt = F16
    nc = bacc.Bacc("TRN2", target_bir_lowering=False, debug=False,
                   num_devices=N_CORES, num_swdge_queues=4)
    x_d = nc.dram_tensor("x_loc", [NPC, H], F32, kind="ExternalInput")
    srcg_d = nc.dram_tensor("srcg", [P, NCH], I32, kind="ExternalInput")
    dstl_d = nc.dram_tensor("dstl", [P, NCH], table_dt, kind="ExternalInput")
    dinvb_d = nc.dram_tensor("dinvb", [P, NBLK], F32, kind="ExternalInput")
    glocb_d = nc.dram_tensor("glocb", [P, NBLK], F32, kind="ExternalInput")
    invcnt_d = nc.dram_tensor("invcnt", [P, GB], F32, kind="ExternalInput")
    W_d = nc.dram_tensor("Wsb", [H, 4 * H], F32, kind="ExternalInput")
    brep_d = nc.dram_tensor("brep", [P, 4 * H], F32, kind="ExternalInput")
    srep_d = nc.dram_tensor("srep", [P, 4 * H], F32, kind="ExternalInput")
    b2rep_d = nc.dram_tensor("b2rep", [P, 4 * H], F32, kind="ExternalInput")
    iota16_d = nc.dram_tensor("iota16", [P, P], table_dt, kind="ExternalInput")
    iota32_d = nc.dram_tensor("iota32", [P, P], F32, kind="ExternalInput")
    hW1_d = nc.dram_tensor("hW1", [H, H], F32, kind="ExternalInput")
    hb1rep_d = nc.dram_tensor("hb1rep", [P, H], F32, kind="ExternalInput")
    hW2_d = nc.dram_tensor("hW2", [H, 1], F32, kind="ExternalInput")
    out_d = nc.dram_tensor("out", [GPC, 1], F32, kind="ExternalOutput")

    NPQ = NPC // GB
    t_loc = [[nc.dram_tensor(f"t_loc{l}_{q}", [NPQ, H], table_dt)
              for q in range(GB)] for l in range(4)]
    T_full = [nc.dram_tensor(f"T_full{l}", [N_CORES * NPC, H], table_dt)
              for l in range(4)]

    with tile.TileContext(nc) as tc:
        with (
            tc.tile_pool(name="persist", bufs=1) as pp,
            tc.tile_pool(name="stagea", bufs=3) as sap,
            tc.tile_pool(name="pool2", bufs=2) as wp2,
            tc.tile_pool(name="psum_agg", bufs=1, space="PSUM") as psagg_tp,
            tc.tile_pool(name="psum_a", bufs=1, space="PSUM") as psa_tp,
            tc.tile_pool(name="psum_p", bufs=1, space="PSUM") as psp_tp,
        ):
            h_sb = pp.tile([P, NBLK * H], F32)
            t_sb = pp.tile([P, NBLK * H], table_dt)
            srcg = pp.tile([P, NCH], I32)
            dstl = pp.tile([P, NCH], table_dt)
            dinvb = pp.tile([P, NBLK], F32)
            glocb = pp.tile([P, NBLK], F32)
            invcnt = pp.tile([P, GB], F32)
            W_sb = pp.tile([H, 4 * H], F32)
            brep = pp.tile([P, 4 * H], F32)
            srep = pp.tile([P, 4 * H], F32)
            b2rep = pp.tile([P, 4 * H], F32)
            iota16 = pp.tile([P, P], table_dt)
            iota32 = pp.tile([P, P], F32)
            hW1_sb = pp.tile([H, H], F32)
            hb1rep = pp.tile([P, H], F32)
            hW2_sb = pp.tile([H, 1], F32)
            ident = pp.tile([P, P], F32)
            ident16 = pp.tile([P, P], table_dt)
            z2all = pp.tile([1, GPC], F32)
            for sb, d in [(srcg, srcg_d), (dstl, dstl_d), (dinvb, dinvb_d),
                          (glocb, glocb_d), (invcnt, invcnt_d), (W_sb, W_d),
                          (brep, brep_d), (srep, srep_d), (b2rep, b2rep_d),
                          (iota16, iota16_d), (iota32, iota32_d),
                          (hW1_sb, hW1_d), (hb1rep, hb1rep_d), (hW2_sb, hW2_d)]:
                nc.sync.dma_start(sb[:], d[:])
            make_identity(nc, ident[:])
            nc.vector.tensor_copy(ident16[:], ident[:])
            for b in range(NBLK):
                nc.sync.dma_start(h_sb[:, b * H:(b + 1) * H],
                                  x_d[b * P:(b + 1) * P, :])

            # chunk buffer sets: [half][k]
            ohset = [[pp.tile([P, P], table_dt, name=f"oh{h}_{k}")
                      for k in range(UNROLL)] for h in range(NW)]
            gset = [[pp.tile([P, H], table_dt, name=f"g{h}_{k}")
                     for k in range(UNROLL)] for h in range(NW)]
            ps_half = [psagg_tp.tile([P, H], F32, space="PSUM", name=f"psagg{h}")
                       for h in range(NW)]
            def agg_chunk(col, half, j, ps, start, stop, T_l):
                g = gset[half][j % UNROLL]
                oh = ohset[half][j % UNROLL]
                inst = nc.gpsimd.indirect_dma_start(
                    out=g[:], out_offset=None, in_=T_l[:],
                    in_offset=bass.IndirectOffsetOnAxis(ap=srcg[:, col:col + 1],
                                                        axis=0))
                if half % 4:
                    inst.ins.queue = f"qPoolDynamic{half % 4}" 
                nc.vector.tensor_tensor(
                    out=oh[:], in0=dstl[:, col:col + 1].to_broadcast([P, P]),
                    in1=iota16[:], op=mybir.AluOpType.is_equal)
                nc.tensor.matmul(ps[:], lhsT=oh[:], rhs=g[:],
                                 start=start, stop=stop, skip_group_check=True)

            def emit_t_block(l, b):
                # tt_l[block b] = dinv * (h[block b] @ W_l), into t_loc[l]
                ls_t = slice(l * H, (l + 1) * H)
                trp = psa_tp.tile([P, H], F32, space="PSUM", name="trp")
                nc.tensor.transpose(out=trp[:],
                                    in_=h_sb[:, b * H:(b + 1) * H],
                                    identity=ident[:])
                hT = sap.tile([P, H], F32, name="hT")
                nc.scalar.copy(hT[:], trp[:])
                tps = psa_tp.tile([P, H], F32, space="PSUM", name="tps")
                nc.tensor.matmul(tps[:], lhsT=hT[:], rhs=W_sb[:, ls_t],
                                 start=True, stop=True, skip_group_check=True)
                nc.scalar.activation(t_sb[:, b * H:(b + 1) * H], tps[:],
                                     mybir.ActivationFunctionType.Copy,
                                     scale=dinvb[:, b:b + 1])
                q, bq = divmod(b, NBLK // GB)
                nc.sync.dma_start(t_loc[l][q][bq * P:(bq + 1) * P, :],
                                  t_sb[:, b * H:(b + 1) * H])

            C2b = NBLK // GB   # blocks per quarter

            def emit_ag(l, q):
                nc.gpsimd.collective_compute(
                    "AllGather", mybir.AluOpType.bypass,
                    replica_groups=[list(range(N_CORES))],
                    ins=[t_loc[l][q][:]],
                    outs=[T_full[l][q * N_CORES * NPQ:
                                    (q + 1) * N_CORES * NPQ, :]])

            with nc.named_scope("stageA0"):
                nq = 0
                for b in range(NBLK):
                    emit_t_block(0, b)
                    while nq < GB and b >= (nq + 1) * C2b - 1:
                        emit_ag(0, nq)
                        nq += 1

            for l in range(4):
                ls = slice(l * H, (l + 1) * H)
                with nc.named_scope(f"agg{l}"):
                    nq = 0
                    for bp in range(NBLK // NW):
                        blocks = [NW * bp + i for i in range(NW)]
                        cb = [C_b[b] for b in blocks]
                        for half in range(NW):
                            b = blocks[half]
                            # self-loop term: tt rows are local and consecutive
                            nc.tensor.matmul(
                                ps_half[half][:], lhsT=ident16[:],
                                rhs=t_sb[:, b * H:(b + 1) * H],
                                start=True, stop=False, skip_group_check=True)
                        for j in range(max(cb)):
                            for half in range(NW):
                                if j < cb[half]:
                                    agg_chunk(col_base[blocks[half]] + j,
                                              half, j, ps_half[half],
                                              False, j == cb[half] - 1,
                                              T_full[l])
                        for half in range(NW):
                            # epilogue: h = BN(relu(dinv*psum + b))
                            b = blocks[half]
                            ps = ps_half[half]
                            e0 = wp2.tile([P, H], F32, name=f"e0_{half}")
                            e1 = wp2.tile([P, H], F32, name=f"e1_{half}")
                            nc.vector.tensor_scalar(
                                e0[:], ps[:], dinvb[:, b:b + 1], None,
                                mybir.AluOpType.mult)
                            nc.vector.tensor_tensor(
                                out=e1[:], in0=e0[:], in1=brep[:, ls],
                                op=mybir.AluOpType.add)
                            nc.scalar.activation(
                                e0[:], e1[:], mybir.ActivationFunctionType.Relu)
                            nc.vector.tensor_tensor(
                                out=e1[:], in0=e0[:], in1=srep[:, ls],
                                op=mybir.AluOpType.mult)
                            nc.vector.tensor_tensor(
                                out=h_sb[:, b * H:(b + 1) * H], in0=e1[:],
                                in1=b2rep[:, ls],
                                op=mybir.AluOpType.add)
                            if l < 3:
                                emit_t_block(l + 1, b)
                        if l < 3:
                            while nq < GB and NW * bp + NW - 1 >= (nq + 1) * C2b - 1:
                                emit_ag(l + 1, nq)
                                nq += 1

            # ---- global mean pool + head
            with nc.named_scope("pool"):
                for gb in range(GB):
                    pps = psp_tp.tile([P, H], F32, space="PSUM", name="pA")
                    for k in range(C2):
                        b = gb * C2 + k
                        oh32 = wp2.tile([P, P], F32, name="oh32")
                        nc.vector.tensor_tensor(
                            out=oh32[:],
                            in0=glocb[:, b:b + 1].to_broadcast([P, P]),
                            in1=iota32[:], op=mybir.AluOpType.is_equal)
                        nc.tensor.matmul(pps[:], lhsT=oh32[:],
                                         rhs=h_sb[:, b * H:(b + 1) * H],
                                         start=(k == 0), stop=(k == C2 - 1),
                                         skip_group_check=True)
                    pooled = wp2.tile([P, H], F32, name="pooled")
                    nc.vector.tensor_scalar(pooled[:], pps[:],
                                            invcnt[:, gb:gb + 1], None,
                                            mybir.AluOpType.mult)
                    # head: relu(pooled @ hW1 + hb1) @ hW2 + hb2
                    trp = psp_tp.tile([P, H], F32, space="PSUM", name="pB")
                    nc.tensor.transpose(out=trp[:], in_=pooled[:],
                                        identity=ident[:])
                    poolT = wp2.tile([P, H], F32, name="poolT")
                    nc.scalar.copy(poolT[:], trp[:])
                    z1ps = psp_tp.tile([P, H], F32, space="PSUM", name="pA")
                    nc.tensor.matmul(z1ps[:], lhsT=poolT[:], rhs=hW1_sb[:],
                                     start=True, stop=True,
                                     skip_group_check=True)
                    r1 = wp2.tile([P, H], F32, name="r1")
                    nc.vector.tensor_tensor(out=r1[:], in0=z1ps[:],
                                            in1=hb1rep[:],
                                            op=mybir.AluOpType.add)
                    nc.scalar.activation(r1[:], r1[:],
                                         mybir.ActivationFunctionType.Relu)
                    tr2 = psp_tp.tile([P, H], F32, space="PSUM", name="pB")
                    nc.tensor.transpose(out=tr2[:], in_=r1[:], identity=ident[:])
                    r1T = wp2.tile([P, H], F32, name="r1T")
                    nc.scalar.copy(r1T[:], tr2[:])
                    z2full = psp_tp.tile([P, P], F32, space="PSUM", name="pA")
                    z2ps = z2full[0:1, :]
                    nc.tensor.matmul(z2ps[:], lhsT=hW2_sb[:], rhs=r1T[:],
                                     start=True, stop=True,
                                     skip_group_check=True)
                    nc.vector.tensor_scalar(
                        z2all[0:1, gb * P:(gb + 1) * P], z2ps[:],
                        float(hb2_val), None, mybir.AluOpType.add)
                nc.sync.dma_start(out_d[:, 0:1], z2all[0:1, :])

    nc.compile()
    return nc


def kernel(**inputs):
    global LAST_EXEC_NS
    x = np.ascontiguousarray(np.asarray(inputs["x"], dtype=np.float32))
    ei = np.asarray(inputs["edge_index"]).astype(np.int64)
    batch = np.asarray(inputs["batch"]).astype(np.int64)
    Ws = np.asarray(inputs["Ws"], dtype=np.float32)
    bs = np.asarray(inputs["bs"], dtype=np.float32)
    gammas = np.asarray(inputs["gammas"], dtype=np.float32)
    betas = np.asarray(inputs["betas"], dtype=np.float32)
    bn_means = np.asarray(inputs["bn_means"], dtype=np.float32)
    bn_vars = np.asarray(inputs["bn_vars"], dtype=np.float32)
    hW1 = np.asarray(inputs["hW1"], dtype=np.float32)
    hb1 = np.asarray(inputs["hb1"], dtype=np.float32)
    hW2 = np.asarray(inputs["hW2"], dtype=np.float32)
    hb2 = np.asarray(inputs["hb2"], dtype=np.float32)

    src, dst = ei[0], ei[1]
    N = x.shape[0]
    deg = np.bincount(dst, minlength=N).astype(np.float64) + 1.0
    dinv = (1.0 / np.sqrt(deg)).astype(np.float32)

    meta = _preprocess(x, src, dst, batch, dinv)
    C2, NBLK, NPC, C_b, NCH = (meta[k] for k in
                               ("C2", "NBLK", "NPC", "C_b", "NCH"))

    key = (C2, NBLK, NPC, C_b, NCH, float(hb2[0]))
    if key not in _CACHE:
        _CACHE[key] = _build(C2, NBLK, NPC, C_b, NCH, float(hb2[0]))
    nc = _CACHE[key]

    # replicated constant arrays
    s_l = gammas / np.sqrt(bn_vars + BN_EPS)            # [4, H]
    b2_l = betas - bn_means * s_l                        # [4, H]
    Wsb = np.ascontiguousarray(Ws.transpose(1, 0, 2).reshape(H, 4 * H))
    brep = np.broadcast_to(bs.reshape(1, 4 * H), (P, 4 * H)).copy()
    srep = np.broadcast_to(s_l.reshape(1, 4 * H), (P, 4 * H)).copy()
    b2rep = np.broadcast_to(b2_l.reshape(1, 4 * H), (P, 4 * H)).copy()
    iota16 = np.broadcast_to(np.arange(P, dtype=np.float16)[None, :],
                             (P, P)).copy()
    iota32 = iota16.astype(np.float32)
    hb1rep = np.broadcast_to(hb1[None, :], (P, H)).copy()

    in_maps = []
    for c in range(N_CORES):
        in_maps.append({
            "x_loc": meta["x_loc"][c],
            "srcg": meta["srcg"][c],
            "dstl": meta["dstl"][c].astype(np.float16),
            "dinvb": meta["dinvb"][c],
            "glocb": meta["glocb"][c],
            "invcnt": meta["invcnt"][c],
            "Wsb": Wsb, "brep": brep, "srep": srep, "b2rep": b2rep,
            "iota16": iota16, "iota32": iota32,
            "hW1": hW1, "hb1rep": hb1rep, "hW2": hW2,
        })

    trace = os.environ.get("BASS_GCN_TRACE", "") == "1"
    if trace:
        bass_utils.upload_artifacts = lambda tmpdir: "local://" + tmpdir
        try:
            import sys, types
            if "antenv.axon_hooks" not in sys.modules:
                mod = types.ModuleType("antenv.axon_hooks")
                _h = [None]
                mod.set_axon_ntff_profile_hook = lambda h: _h.__setitem__(0, h)
                mod.get_axon_ntff_profile_hook = lambda: _h[0]
                sys.modules["antenv.axon_hooks"] = mod
                import antenv
                antenv.axon_hooks = mod
                from trn_agent_boot.trn_boot import _ntff_profile_via_ctypes
                mod.set_axon_ntff_profile_hook(
                    _ntff_profile_via_ctypes("/opt/axon/libaxon_pjrt.so"))
        except Exception as e:
            print(f"NTFF hook registration failed: {e}")
    res = bass_utils.run_bass_kernel_spmd(nc, in_maps, list(range(N_CORES)),
                                          trace=trace)
    LAST_EXEC_NS = res.exec_time_ns
    if res.exec_time_ns is not None:
        print(f"HW exec time: {res.exec_time_ns} ns")

    out = np.concatenate([res.results[c]["out"] for c in range(N_CORES)],
                         axis=0).astype(np.float32)
    return out



# revision 16
# speedup vs baseline: 1.9774x; 1.9774x over previous
"""GCN (4x GCNConv + eval BN + ReLU, global mean pool, 2-layer MLP head) on 8
Trainium2 NeuronCores via Bass/Tile.

Sharding: data-parallel over graphs. 4096 graphs -> 8 cores x 512 contiguous
graphs (batch is sorted). Within a core the 512 graphs form 4 pool groups of
128 graphs; each group's nodes are padded to a multiple of 128 rows so pooling
blocks align with node blocks. Edges live on the core owning their dst node.

Per layer (all on device):
  tt = dinv * (h_local @ W_l)           per-core shard, f16 table
  AllGather tt across the 8 cores       (the only collective)
  agg[v] = dinv[v] * sum_{e: dst=v} tt[src_e]   with a weighted-identity
                                                matmul for the self-loop term
  h = BN_l(relu(agg + b_l))
The segment-sum runs as one-hot matmuls. Key design points:

* Edge rows are fetched with InstDMAGatherAnt (gpsimd.dma_gather): one
  instruction gathers ~2K arbitrary table rows by an int16 index list, so
  SWDGE descriptor generation (994ns fixed + 0.34ns/row, serialized on the
  Pool engine) is amortized over whole 4-block groups. The baseline's
  one-indirect-DMA-per-128-edge-chunk put 7.4ms of SWDGE on the Pool engine.
  int16 indices only reach 32K rows, so gathers are split by table QUARTER
  (26624 rows), which also lets a quarter's chunks start right after that
  quarter's AllGather lands. Indices are wrapped into 16 partitions and
  replicated across the 8 GPSIMD stripes (HW contract).
* The one-hot scatter matrices for all chunks of a gather are built in one
  DVE op via 3D access patterns, then scaled by per-edge dst weights
  (dinv[dst_e]) in a second op, folding the symmetric normalization into the
  scatter matmul. Chunk padding slots carry weight 0 (and index 0).
* Layers 0-2 run the scatter matmul "flipped": lhsT = gathered rows
  (stationary), rhs = one-hot (moving), producing agg TRANSPOSED [h, node] in
  PSUM. The BN+ReLU epilogue then has per-PARTITION constants (one scalar
  activation op), and the next layer's h @ W matmul consumes aggT directly as
  lhsT -- no transposes anywhere in the steady state. Layer 3 runs in the
  original orientation so pooling sees node-major h.
* h and W are bf16 (table stays f16); epilogue relu on the Scalar engine.

All data-dependent structure is precomputed host-side into per-core meta
arrays; the chunk layout is maxed over cores so the device program is
identical across cores (SPMD).
"""

import os
import numpy as np

import concourse.bass as bass
import concourse.tile as tile
from concourse import mybir, bacc, bass_utils
from concourse.masks import make_identity

P = 128
H = 128
N_CORES = 8
N_GRAPHS = 4096
GPC = N_GRAPHS // N_CORES      # graphs per core
GB = 4                         # pool groups (of 128 graphs) per core
NQ = 4                         # table quarters (int16 index range)
BN_EPS = 1e-5
NW = 4                         # blocks per gather group / PSUM streams

F32 = mybir.dt.float32
F16 = mybir.dt.float16
BF16 = mybir.dt.bfloat16
I32 = mybir.dt.int32
I16 = mybir.dt.int16

LAST_EXEC_NS = None
_CACHE = {}


def _preprocess(x, src, dst, batch, dinv):
    """Host-side sharding: node remap + per-core padded meta arrays."""
    N = x.shape[0]
    graph_start = np.searchsorted(batch, np.arange(N_GRAPHS + 1))
    seg_rows = np.zeros((N_CORES, GB), dtype=np.int64)
    for c in range(N_CORES):
        for g in range(GB):
            g0 = c * GPC + g * P
            seg_rows[c, g] = graph_start[g0 + P] - graph_start[g0]
    C2 = int(np.ceil(seg_rows.max() / P))     # node blocks per pool group
    NBLK = GB * C2                            # node blocks per core
    NPC = NBLK * P                            # padded nodes per core
    NGRP = (NBLK + NW - 1) // NW

    newid = np.zeros(N, dtype=np.int64)
    for c in range(N_CORES):
        for g in range(GB):
            g0 = c * GPC + g * P
            r0, r1 = graph_start[g0], graph_start[g0 + P]
            newid[r0:r1] = c * NPC + g * C2 * P + np.arange(r1 - r0)

    xT_loc = np.zeros((N_CORES, H, NPC), dtype=np.float32)
    dinvb = np.ones((N_CORES, P, NBLK), dtype=np.float32)
    glocb = np.full((N_CORES, P, NBLK), -1.0, dtype=np.float32)
    invcnt = np.ones((N_CORES, P, GB), dtype=np.float32)
    loc_all = newid % NPC
    core_all = newid // NPC
    for c in range(N_CORES):
        m = core_all == c
        loc = loc_all[m]
        xT_loc[c][:, loc] = x[m].T
        dinvb[c, loc % P, loc // P] = dinv[m]
        gl = (batch[m] - c * GPC).astype(np.int64)      # 0..GPC-1
        glocb[c, loc % P, loc // P] = (gl % P).astype(np.float32)
        cnt = np.zeros(GPC, dtype=np.float64)
        np.add.at(cnt, gl, 1.0)
        invcnt[c] = (1.0 / np.maximum(cnt, 1.0)).reshape(GB, P).T.astype(np.float32)

    # edges grouped by (4-block group, src quarter, dst block); self-loops
    # handled by weighted-identity matmuls on device. table rows live in
    # [quarter][core][row] order (quarter AllGathers).
    NPQ = NPC // GB
    QRNG = N_CORES * NPQ                      # rows per table quarter
    def table_row(gid):
        c = gid // NPC
        i = gid % NPC
        return (i // NPQ) * QRNG + c * NPQ + (i % NPQ)
    e_src_g = table_row(newid[src])
    e_q = e_src_g // QRNG
    e_ridx = (e_src_g % QRNG).astype(np.int16)
    e_dst_core = core_all[dst]
    e_dst_loc = loc_all[dst]
    e_dst_w = dinv[dst]

    NK = NGRP * NQ * NBLK
    e_blk = e_dst_loc // P
    e_key = ((e_blk // NW) * NQ + e_q) * NBLK + e_blk
    # chunk counts maxed over cores so the SPMD program is shared
    cnt_k = np.zeros(NK, dtype=np.int64)
    for c in range(N_CORES):
        cc = np.bincount(e_key[e_dst_core == c], minlength=NK)
        np.maximum(cnt_k, cc, out=cnt_k)
    C_k = -(-cnt_k // P)                      # ceil; 0 where no edges anywhere
    colbase_k = np.concatenate([[0], np.cumsum(C_k)])
    NCHQ = int(colbase_k[-1])

    dstl = np.full((N_CORES, P, NCHQ), -1.0, dtype=np.float32)
    dstw = np.zeros((N_CORES, P, NCHQ), dtype=np.float32)
    idx16 = np.zeros((N_CORES, P, 8 * NCHQ), dtype=np.int16)
    first_k = np.concatenate([[0], np.cumsum(cnt_k)])  # unused; per-core below
    for c in range(N_CORES):
        m = e_dst_core == c
        key = e_key[m]
        order = np.argsort(key, kind="stable")
        key = key[order]
        slot = (e_dst_loc[m] % P)[order]
        ridx = e_ridx[m][order]
        w = e_dst_w[m][order]
        cnt_c = np.bincount(key, minlength=NK)
        start_c = np.concatenate([[0], np.cumsum(cnt_c)])
        i_in_seg = np.arange(len(key)) - start_c[key]
        col = colbase_k[key] + i_in_seg // P
        p = i_in_seg % P
        dstl[c, p, col] = slot.astype(np.float32)
        dstw[c, p, col] = w
        wrapped = np.zeros((16, 8 * NCHQ), dtype=np.int16)
        wrapped[p % 16, 8 * col + p // 16] = ridx
        idx16[c] = wrapped[np.arange(P) % 16, :]

    C_gqb = C_k.reshape(NGRP, NQ, NBLK)
    colbase_gqb = colbase_k[:-1].reshape(NGRP, NQ, NBLK)

    return dict(C2=C2, NBLK=NBLK, NPC=NPC, NCHQ=NCHQ, NGRP=NGRP,
                C_gqb=C_gqb, colbase_gqb=colbase_gqb,
                xT_loc=xT_loc, dinvb=dinvb, glocb=glocb, invcnt=invcnt,
                idx16=idx16, dstl=dstl, dstw=dstw)


def _build(C2, NBLK, NPC, NCHQ, NGRP, C_gqb, colbase_gqb, hb2_val,
           debug=False):
    JMAXQ = int(C_gqb.sum(axis=2).max())  # buffer size: chunks per (gg, q)
    JCAP = 8   # chunks per dma_gather instr: 1024 idxs = SWDGE ring capacity
    table_dt = F16
    nc = bacc.Bacc("TRN2", target_bir_lowering=False, debug=False,
                   num_devices=N_CORES, num_swdge_queues=4)
    xT_d = nc.dram_tensor("xT_loc", [H, NPC], BF16, kind="ExternalInput")
    idx16_d = nc.dram_tensor("idx16", [P, 8 * NCHQ], I16,
                             kind="ExternalInput")
    dstl_d = nc.dram_tensor("dstl", [P, NCHQ], table_dt, kind="ExternalInput")
    dstw_d = nc.dram_tensor("dstw", [P, NCHQ], table_dt, kind="ExternalInput")
    dinvb_d = nc.dram_tensor("dinvb", [P, NBLK], F32, kind="ExternalInput")
    glocb_d = nc.dram_tensor("glocb", [P, NBLK], F32, kind="ExternalInput")
    invcnt_d = nc.dram_tensor("invcnt", [P, GB], F32, kind="ExternalInput")
    W_d = nc.dram_tensor("Wsb", [H, 4 * H], BF16, kind="ExternalInput")
    scol_d = nc.dram_tensor("scol", [P, 4], F32, kind="ExternalInput")
    sbcol_d = nc.dram_tensor("sbcol", [P, 4], F32, kind="ExternalInput")
    b2col_d = nc.dram_tensor("b2col", [P, 4], F32, kind="ExternalInput")
    srep3_d = nc.dram_tensor("srep3", [P, H], F32, kind="ExternalInput")
    sbrep3_d = nc.dram_tensor("sbrep3", [P, H], F32, kind="ExternalInput")
    b2rep3_d = nc.dram_tensor("b2rep3", [P, H], F32, kind="ExternalInput")
    iota16_d = nc.dram_tensor("iota16", [P, P], table_dt, kind="ExternalInput")
    iota32_d = nc.dram_tensor("iota32", [P, P], F32, kind="ExternalInput")
    hW1_d = nc.dram_tensor("hW1", [H, H], F32, kind="ExternalInput")
    hb1rep_d = nc.dram_tensor("hb1rep", [P, H], F32, kind="ExternalInput")
    hW2_d = nc.dram_tensor("hW2", [H, 1], F32, kind="ExternalInput")
    out_d = nc.dram_tensor("out", [GPC, 1], F32, kind="ExternalOutput")
    hd_d = [nc.dram_tensor(f"hdump{l}", [P, NBLK * H], F32,
                           kind="ExternalOutput")
            for l in range(4)] if debug else None
    td_d = (nc.dram_tensor("tdump", [P, NBLK * H], F32,
                           kind="ExternalOutput") if debug else None)

    NPQ = NPC // GB
    QRNG = N_CORES * NPQ
    t_loc = [[nc.dram_tensor(f"t_loc{l}_{q}", [NPQ, H], table_dt)
              for q in range(GB)] for l in range(4)]
    T_full = [nc.dram_tensor(f"T_full{l}", [N_CORES * NPC, H], table_dt)
              for l in range(4)]

    with tile.TileContext(nc) as tc:
        with (
            tc.tile_pool(name="persist", bufs=1) as pp,
            tc.tile_pool(name="stagea", bufs=3) as sap,
            tc.tile_pool(name="stream", bufs=2) as sp,
            tc.tile_pool(name="pool2", bufs=2) as wp2,
            tc.tile_pool(name="psum_agg", bufs=1, space="PSUM") as psagg_tp,
            tc.tile_pool(name="psum_a", bufs=2, space="PSUM") as psa_tp,
            tc.tile_pool(name="psum_p", bufs=1, space="PSUM") as psp_tp,
        ):
            h_sb = pp.tile([P, NBLK * H], BF16)
            t_sb = pp.tile([P, NBLK * H], table_dt)
            idx16 = pp.tile([P, 8 * NCHQ], I16)
            dstl = pp.tile([P, NCHQ], table_dt)
            dstw = pp.tile([P, NCHQ], table_dt)
            dinvb = pp.tile([P, NBLK], F32)
            glocb = pp.tile([P, NBLK], F32)
            invcnt = pp.tile([P, GB], F32)
            W_sb = pp.tile([H, 4 * H], BF16)
            scol = pp.tile([P, 4], F32)
            sbcol = pp.tile([P, 4], F32)
            b2col = pp.tile([P, 4], F32)
            srep3 = pp.tile([P, H], F32)
            sbrep3 = pp.tile([P, H], F32)
            b2rep3 = pp.tile([P, H], F32)
            iota16 = pp.tile([P, P], table_dt)
            iota32 = pp.tile([P, P], F32)
            hW1_sb = pp.tile([H, H], F32)
            hb1rep = pp.tile([P, H], F32)
            hW2_sb = pp.tile([H, 1], F32)
            ident = pp.tile([P, P], F32)
            ident16 = pp.tile([P, P], table_dt)
            z2all = pp.tile([1, GPC], F32)
            for sb, d in [(idx16, idx16_d), (dstl, dstl_d), (dstw, dstw_d),
                          (dinvb, dinvb_d), (glocb, glocb_d),
                          (invcnt, invcnt_d), (W_sb, W_d),
                          (scol, scol_d), (sbcol, sbcol_d), (b2col, b2col_d),
                          (srep3, srep3_d), (sbrep3, sbrep3_d),
                          (b2rep3, b2rep3_d),
                          (iota16, iota16_d), (iota32, iota32_d),
                          (hW1_sb, hW1_d), (hb1rep, hb1rep_d),
                          (hW2_sb, hW2_d)]:
                nc.sync.dma_start(sb[:], d[:])
            make_identity(nc, ident[:])
            nc.vector.tensor_copy(ident16[:], ident[:])
            nc.sync.dma_start(h_sb[:], xT_d[:])

            ps_st = [psagg_tp.tile([P, P], F32, space="PSUM", name=f"psagg{s}")
                     for s in range(NW)]

            def emit_gather_parts(gg, T_l):
                """Per-quarter gathers (split to fit the SWDGE descriptor
                ring) + one one-hot build per quarter, for a block group."""
                parts = []
                for q in range(NQ):
                    J = int(C_gqb[gg, q].sum())
                    if J == 0:
                        continue
                    c0 = int(colbase_gqb[gg, q, gg * NW])
                    g = sp.tile([P, JMAXQ * H], table_dt, name=f"g{q}")
                    oh = sp.tile([P, JMAXQ * P], table_dt, name=f"oh{q}")
                    gap = g[:]
                    done = 0
                    while done < J:
                        Jp = min(JCAP, J - done)
                        cc = c0 + done
                        out3 = bass.AP(gap.tensor,
                                       gap.offset + done * H,
                                       [gap.ap[0], [H, Jp], [1, H]])
                        nc.gpsimd.dma_gather(
                            out_ap=out3,
                            in_ap=T_l[q * QRNG:(q + 1) * QRNG, :],
                            idxs_ap=idx16[:, 8 * cc:8 * (cc + Jp)],
                            num_idxs=P * Jp,
                            num_idxs_reg=P * Jp,
                            elem_size=H,
                            queue_num=q,
                        )
                        done += Jp
                    oh_ap = oh[:]
                    oh3 = bass.AP(oh_ap.tensor, oh_ap.offset,
                                  [oh_ap.ap[0], [P, J], [1, P]])
                    ia = iota16[:]
                    iota3 = bass.AP(ia.tensor, ia.offset,
                                    [ia.ap[0], [0, J], ia.ap[1]])
                    nc.vector.tensor_tensor(
                        out=oh3,
                        in0=dstl[:, c0:c0 + J].to_broadcast([P, J, P]),
                        in1=iota3, op=mybir.AluOpType.is_equal)
                    nc.vector.tensor_tensor(
                        out=oh3, in0=oh3,
                        in1=dstw[:, c0:c0 + J].to_broadcast([P, J, P]),
                        op=mybir.AluOpType.mult)
                    parts.append((q, c0, J, g, oh))
                return parts

            def emit_t_block(l, b):
                # t_l[block b] = dinv * (hT[block b]^T @ W_l), into t_loc[l]
                # hT block is [h, node]; lhsT = hT -> out [node, h'].
                ls_t = slice(l * H, (l + 1) * H)
                tps = psa_tp.tile([P, H], F32, space="PSUM", name="tps")
                nc.tensor.matmul(tps[:], lhsT=h_sb[:, b * H:(b + 1) * H],
                                 rhs=W_sb[:, ls_t],
                                 start=True, stop=True, skip_group_check=True)
                nc.vector.tensor_scalar(t_sb[:, b * H:(b + 1) * H], tps[:],
                                        dinvb[:, b:b + 1], None,
                                        mybir.AluOpType.mult)
                q, bq = divmod(b, NBLK // GB)
                nc.sync.dma_start(t_loc[l][q][bq * P:(bq + 1) * P, :],
                                  t_sb[:, b * H:(b + 1) * H])
                if debug and l == 0:
                    tf = sap.tile([P, H], F32, name="tdmp")
                    nc.vector.tensor_copy(tf[:], t_sb[:, b * H:(b + 1) * H])
                    nc.sync.dma_start(td_d[:, b * H:(b + 1) * H], tf[:])

            C2b = NBLK // GB   # blocks per pool quarter

            def emit_ag(l, q):
                nc.gpsimd.collective_compute(
                    "AllGather", mybir.AluOpType.bypass,
                    replica_groups=[list(range(N_CORES))],
                    ins=[t_loc[l][q][:]],
                    outs=[T_full[l][q * QRNG:(q + 1) * QRNG, :]])

            with nc.named_scope("stageA0"):
                nq_ = 0
                for b in range(NBLK):
                    emit_t_block(0, b)
                    while nq_ < GB and b >= (nq_ + 1) * C2b - 1:
                        emit_ag(0, nq_)
                        nq_ += 1

            for l in range(4):
                flip = l < 3
                with nc.named_scope(f"agg{l}"):
                    nq_ = 0
                    for gg in range(NGRP):
                        blocks = list(range(gg * NW, min((gg + 1) * NW, NBLK)))
                        parts = emit_gather_parts(gg, T_full[l])
                        rem = {b: int(C_gqb[gg, :, b].sum()) for b in blocks}
                        for st, b in enumerate(blocks):
                            identw = sp.tile([P, P], table_dt,
                                             name=f"idw{st}")
                            nc.vector.tensor_scalar(identw[:], ident16[:],
                                                    dinvb[:, b:b + 1], None,
                                                    mybir.AluOpType.mult)
                            tblk = t_sb[:, b * H:(b + 1) * H]
                            ps = ps_st[st]
                            if flip:
                                nc.tensor.matmul(ps[:], lhsT=tblk,
                                                 rhs=identw[:], start=True,
                                                 stop=(rem[b] == 0),
                                                 skip_group_check=True)
                            else:
                                nc.tensor.matmul(ps[:], lhsT=identw[:],
                                                 rhs=tblk, start=True,
                                                 stop=(rem[b] == 0),
                                                 skip_group_check=True)
                        for (q, cc, Jp, g, oh) in parts:
                            for j in range(Jp):
                                col = cc + j
                                # block of this chunk: walk C_gqb row
                                rel = col - int(colbase_gqb[gg, q, gg * NW])
                                b = None
                                acc = 0
                                for bb in blocks:
                                    nb = int(C_gqb[gg, q, bb])
                                    if rel < acc + nb:
                                        b = bb
                                        break
                                    acc += nb
                                st = b - gg * NW
                                ps = ps_st[st]
                                rem[b] -= 1
                                if flip:
                                    nc.tensor.matmul(
                                        ps[:], lhsT=g[:, j * H:(j + 1) * H],
                                        rhs=oh[:, j * P:(j + 1) * P],
                                        start=False, stop=(rem[b] == 0),
                                        skip_group_check=True)
                                else:
                                    nc.tensor.matmul(
                                        ps[:], lhsT=oh[:, j * P:(j + 1) * P],
                                        rhs=g[:, j * H:(j + 1) * H],
                                        start=False, stop=(rem[b] == 0),
                                        skip_group_check=True)
                        for st, b in enumerate(blocks):
                            ps = ps_st[st]
                            if flip:
                                # h = relu(s*aggT + s*b) + b2, per-partition
                                nc.scalar.activation(
                                    h_sb[:, b * H:(b + 1) * H], ps[:],
                                    mybir.ActivationFunctionType.Relu,
                                    bias=sbcol[:, l:l + 1],
                                    scale=scol[:, l:l + 1])
                                nc.vector.tensor_scalar(
                                    h_sb[:, b * H:(b + 1) * H],
                                    h_sb[:, b * H:(b + 1) * H],
                                    b2col[:, l:l + 1], None,
                                    mybir.AluOpType.add)
                                if debug:
                                    hf = sap.tile([P, H], F32, name="hdmp")
                                    nc.vector.tensor_copy(
                                        hf[:], h_sb[:, b * H:(b + 1) * H])
                                    nc.sync.dma_start(
                                        hd_d[l][:, b * H:(b + 1) * H], hf[:])
                                emit_t_block(l + 1, b)
                            else:
                                e0 = wp2.tile([P, H], F32, name=f"e0_{st}")
                                e1 = wp2.tile([P, H], F32, name=f"e1_{st}")
                                nc.vector.tensor_tensor(
                                    out=e0[:], in0=ps[:], in1=srep3[:],
                                    op=mybir.AluOpType.mult)
                                nc.vector.tensor_tensor(
                                    out=e1[:], in0=e0[:], in1=sbrep3[:],
                                    op=mybir.AluOpType.add)
                                nc.scalar.activation(
                                    e0[:], e1[:],
                                    mybir.ActivationFunctionType.Relu)
                                nc.vector.tensor_tensor(
                                    out=h_sb[:, b * H:(b + 1) * H],
                                    in0=e0[:], in1=b2rep3[:],
                                    op=mybir.AluOpType.add)
                                if debug:
                                    hf = sap.tile([P, H], F32, name="hdmp")
                                    nc.vector.tensor_copy(
                                        hf[:], h_sb[:, b * H:(b + 1) * H])
                                    nc.sync.dma_start(
                                        hd_d[l][:, b * H:(b + 1) * H], hf[:])
                        if flip:
                            last_b = blocks[-1]
                            while nq_ < GB and last_b >= (nq_ + 1) * C2b - 1:
                                emit_ag(l + 1, nq_)
                                nq_ += 1

            # ---- global mean pool + head
            with nc.named_scope("pool"):
                for gb in range(GB):
                    pps = psp_tp.tile([P, H], F32, space="PSUM", name="pA")
                    for k in range(C2):
                        b = gb * C2 + k
                        ohp = wp2.tile([P, P], BF16, name="ohp")
                        nc.vector.tensor_tensor(
                            out=ohp[:],
                            in0=glocb[:, b:b + 1].to_broadcast([P, P]),
                            in1=iota32[:], op=mybir.AluOpType.is_equal)
                        nc.tensor.matmul(pps[:], lhsT=ohp[:],
                                         rhs=h_sb[:, b * H:(b + 1) * H],
                                         start=(k == 0), stop=(k == C2 - 1),
                                         skip_group_check=True)
                    pooled = wp2.tile([P, H], F32, name="pooled")
                    nc.vector.tensor_scalar(pooled[:], pps[:],
                                            invcnt[:, gb:gb + 1], None,
                                            mybir.AluOpType.mult)
                    # head: relu(pooled @ hW1 + hb1) @ hW2 + hb2
                    trp = psp_tp.tile([P, H], F32, space="PSUM", name="pB")
                    nc.tensor.transpose(out=trp[:], in_=pooled[:],
                                        identity=ident[:])
                    poolT = wp2.tile([P, H], F32, name="poolT")
                    nc.scalar.copy(poolT[:], trp[:])
                    z1ps = psp_tp.tile([P, H], F32, space="PSUM", name="pA")
                    nc.tensor.matmul(z1ps[:], lhsT=poolT[:], rhs=hW1_sb[:],
                                     start=True, stop=True,
                                     skip_group_check=True)
                    r1 = wp2.tile([P, H], F32, name="r1")
                    nc.vector.tensor_tensor(out=r1[:], in0=z1ps[:],
                                            in1=hb1rep[:],
                                            op=mybir.AluOpType.add)
                    nc.scalar.activation(r1[:], r1[:],
                                         mybir.ActivationFunctionType.Relu)
                    tr2 = psp_tp.tile([P, H], F32, space="PSUM", name="pB")
                    nc.tensor.transpose(out=tr2[:], in_=r1[:], identity=ident[:])
                    r1T = wp2.tile([P, H], F32, name="r1T")
                    nc.scalar.copy(r1T[:], tr2[:])
                    z2full = psp_tp.tile([P, P], F32, space="PSUM", name="pA")
                    z2ps = z2full[0:1, :]
                    nc.tensor.matmul(z2ps[:], lhsT=hW2_sb[:], rhs=r1T[:],
                                     start=True, stop=True,
                                     skip_group_check=True)
                    nc.vector.tensor_scalar(
                        z2all[0:1, gb * P:(gb + 1) * P], z2ps[:],
                        float(hb2_val), None, mybir.AluOpType.add)
                nc.sync.dma_start(out_d[:, 0:1], z2all[0:1, :])

    nc.compile()
    return nc


def kernel(**inputs):
    global LAST_EXEC_NS
    x = np.ascontiguousarray(np.asarray(inputs["x"], dtype=np.float32))
    ei = np.asarray(inputs["edge_index"]).astype(np.int64)
    batch = np.asarray(inputs["batch"]).astype(np.int64)
    Ws = np.asarray(inputs["Ws"], dtype=np.float32)
    bs = np.asarray(inputs["bs"], dtype=np.float32)
    gammas = np.asarray(inputs["gammas"], dtype=np.float32)
    betas = np.asarray(inputs["betas"], dtype=np.float32)
    bn_means = np.asarray(inputs["bn_means"], dtype=np.float32)
    bn_vars = np.asarray(inputs["bn_vars"], dtype=np.float32)
    hW1 = np.asarray(inputs["hW1"], dtype=np.float32)
    hb1 = np.asarray(inputs["hb1"], dtype=np.float32)
    hW2 = np.asarray(inputs["hW2"], dtype=np.float32)
    hb2 = np.asarray(inputs["hb2"], dtype=np.float32)

    src, dst = ei[0], ei[1]
    N = x.shape[0]
    deg = np.bincount(dst, minlength=N).astype(np.float64) + 1.0
    dinv = (1.0 / np.sqrt(deg)).astype(np.float32)

    meta = _preprocess(x, src, dst, batch, dinv)
    C2, NBLK, NPC, NCHQ, NGRP = (meta[k] for k in
                                 ("C2", "NBLK", "NPC", "NCHQ", "NGRP"))

    debug = os.environ.get("BASS_GCN_DEBUG", "") == "1"
    key = (C2, NBLK, NPC, NCHQ, NGRP,
           tuple(meta["C_gqb"].ravel().tolist()), float(hb2[0]), debug)
    if key not in _CACHE:
        _CACHE[key] = _build(C2, NBLK, NPC, NCHQ, NGRP,
                             meta["C_gqb"], meta["colbase_gqb"],
                             float(hb2[0]), debug=debug)
    nc = _CACHE[key]

    bf16 = mybir.dt.np(BF16)
    # replicated constant arrays
    s_l = gammas / np.sqrt(bn_vars + BN_EPS)            # [4, H]
    b2_l = betas - bn_means * s_l                        # [4, H]
    sb_l = s_l * bs                                      # [4, H]
    Wsb = np.ascontiguousarray(
        Ws.transpose(1, 0, 2).reshape(H, 4 * H)).astype(bf16)
    scol = np.ascontiguousarray(s_l.T)                   # [H, 4]
    sbcol = np.ascontiguousarray(sb_l.T)
    b2col = np.ascontiguousarray(b2_l.T)
    srep3 = np.broadcast_to(s_l[3][None, :], (P, H)).copy()
    sbrep3 = np.broadcast_to(sb_l[3][None, :], (P, H)).copy()
    b2rep3 = np.broadcast_to(b2_l[3][None, :], (P, H)).copy()
    iota16 = np.broadcast_to(np.arange(P, dtype=np.float16)[None, :],
                             (P, P)).copy()
    iota32 = iota16.astype(np.float32)
    hb1rep = np.broadcast_to(hb1[None, :], (P, H)).copy()

    in_maps = []
    for c in range(N_CORES):
        in_maps.append({
            "xT_loc": meta["xT_loc"][c].astype(bf16),
            "idx16": meta["idx16"][c],
            "dstl": meta["dstl"][c].astype(np.float16),
            "dstw": meta["dstw"][c].astype(np.float16),
            "dinvb": meta["dinvb"][c],
            "glocb": meta["glocb"][c],
            "invcnt": meta["invcnt"][c],
            "Wsb": Wsb, "scol": scol, "sbcol": sbcol, "b2col": b2col,
            "srep3": srep3, "sbrep3": sbrep3, "b2rep3": b2rep3,
            "iota16": iota16, "iota32": iota32,
            "hW1": hW1, "hb1rep": hb1rep, "hW2": hW2,
        })

    trace = os.environ.get("BASS_GCN_TRACE", "") == "1"
    if trace:
        bass_utils.upload_artifacts = lambda tmpdir: "local://" + tmpdir
        try:
            import sys, types
            if "antenv.axon_hooks" not in sys.modules:
                mod = types.ModuleType("antenv.axon_hooks")
                _h = [None]
                mod.set_axon_ntff_profile_hook = lambda h: _h.__setitem__(0, h)
                mod.get_axon_ntff_profile_hook = lambda: _h[0]
                sys.modules["antenv.axon_hooks"] = mod
                import antenv
                antenv.axon_hooks = mod
                from trn_agent_boot.trn_boot import _ntff_profile_via_ctypes
                mod.set_axon_ntff_profile_hook(
                    _ntff_profile_via_ctypes("/opt/axon/libaxon_pjrt.so"))
        except Exception as e:
            print(f"NTFF hook registration failed: {e}")
    res = bass_utils.run_bass_kernel_spmd(nc, in_maps, list(range(N_CORES)),
                                          trace=trace)
    LAST_EXEC_NS = res.exec_time_ns
    if res.exec_time_ns is not None:
        print(f"HW exec time: {res.exec_time_ns} ns")

    if debug:
        kernel.DEBUG_RES = res.results
        kernel.DEBUG_META = meta
    out = np.concatenate([res.results[c]["out"] for c in range(N_CORES)],
                         axis=0).astype(np.float32)
    return out
